# revision 5
# baseline (speedup 1.0000x reference)
"""Trainium2 Bass kernel for LocalBackwardTemporalAttention (optimized v2).

Data-parallel over batch B=8 (one element per core, no collectives).
Single fused on-chip pipeline; SBUF-resident intermediates with
tag-chained slot reuse (two sequential resident pools res1/res2):

  A: LN(x) -> kv_fm (SBUF fm bf16), kv_tm -> DRAM; q_ln -> SBUF
  B: QKV GEMMs from kv_fm -> k_fm, v_res (per-t token-major), q_fm
  C: attention per (hp, t-pair, pi): S = q@k^T (no max-sub) -> exp ->
     sum/recip/scale -> PE-transpose -> attnT@v -> o_fm (SBUF)
  D: out_proj o_fm -> att_fm (SBUF)
  E1: mlpq GEMM1 (+gelu), M-halved weights -> h1q (DRAM)
  --- res1 -> res2 ---
  E2+F: per 128-token chunk: GEMM2 swapped + b2q + kv_tm residual
        -> res_ln -> ln2 -> PE-transpose -> ln2t_fm (SBUF)
  G1: mlp GEMM1 (+gelu), M-halved -> h1 (DRAM)
  G2: GEMM2 swapped + b2 -> out (DRAM, token-major f32)

All GEMMs bf16 in / f32 PSUM accum. DRAM round trips only for kv_tm,
h1q, h1. DMA transfers are 0.125-4 MB (vs ~1100 tiny DMAs in v1).
"""

import sys

sys.path.insert(0, "/opt/trn_rl_repo")

import numpy as np
import ml_dtypes

import concourse.bass as bass
import concourse.bacc as bacc_mod
import concourse.mybir as mybir
import concourse.tile as tile
from concourse.masks import make_identity

F32 = mybir.dt.float32
BF16 = mybir.dt.bfloat16
AF = mybir.ActivationFunctionType
ALU = mybir.AluOpType
AX = mybir.AxisListType

B, HW, NF, E, M, H = 8, 16, 196, 1024, 4096, 16
T, D = HW - 1, E // H            # 15, 64
NKV, NQ, NTOK = T * NF, NF, HW * NF
EPS = 1e-6
P = 128
NT = 490                         # fm GEMM token tile (2940 = 6*490)
KE, KM = E // P, M // P          # 8, 32
EH = E // 2                      # 512
NTILES = NKV // NT               # 6
MH = M // 2                      # 2048


def _ceil(a, b):
    return -(-a // b)


def _bcast_ap(handle, n):
    a = handle[:]
    return bass.AP(tensor=a.tensor, offset=a.offset, ap=[[0, n], list(a.ap[0])])


def _col_ap(handle, mo):
    a = handle[:]
    return bass.AP(tensor=a.tensor, offset=a.offset, ap=[[1, P], [P, mo]])


def build_nc():
    nc = bacc_mod.Bacc(None, target_bir_lowering=False)
    t = lambda n, s, d: nc.dram_tensor(n, s, d, kind="ExternalInput")

    x = t("x", [NTOK, E], F32)
    wqT = t("wqT", [E, E], BF16)
    wkT = t("wkT", [E, E], BF16)
    wvT = t("wvT", [E, E], BF16)
    woT = t("woT", [E, E], BF16)
    w1qT = t("w1qT", [E, M], BF16)
    w2qT = t("w2qT", [M, E], BF16)
    w1T = t("w1T", [E, M], BF16)
    w2T = t("w2T", [M, E], BF16)
    bqs = t("bqs", [E], F32)
    bk = t("bk", [E], F32)
    bv = t("bv", [E], F32)
    bo = t("bo", [E], F32)
    b1q = t("b1q", [M], F32)
    b2q = t("b2q", [E], F32)
    b1 = t("b1", [M], F32)
    b2 = t("b2", [E], F32)
    gq = t("gq", [E], F32)
    bbq = t("bbq", [E], F32)
    gkv = t("gkv", [E], F32)
    bbkv = t("bbkv", [E], F32)
    gres = t("gres", [E], F32)
    bbres = t("bbres", [E], F32)
    gln2 = t("gln2", [E], F32)
    bbln2 = t("bbln2", [E], F32)
    out = nc.dram_tensor("out", [NKV, E], F32, kind="ExternalOutput")

    with tile.TileContext(nc) as tc:
        with tc.tile_pool(name="dram", bufs=1, space="DRAM") as dram, \
             tc.tile_pool(name="consts", bufs=1) as consts:
            kv_tm = dram.tile([NKV, E], BF16)
            h1q_nt = dram.tile([NTILES, P, KM, NT], BF16)
            h1_nt = dram.tile([NTILES, P, KM, NT], BF16)

            ident = consts.tile([P, P], BF16)
            make_identity(nc, ident)
            epst = consts.tile([P, 1], F32)
            nc.vector.memset(epst, EPS)

            # =================== res1: stages A..E1 ===================
            with tc.tile_pool(name="res1", bufs=1) as res1:
                # tagA: kv_fm -> o_fm          (47 KB/part)
                # tagB: k_fm -> att_fm         (47 KB)
                # tagC: v_res -> w1q halves    (60 KB)
                # tagD: qln_fm -> wo           (16 KB)
                # tagE: q_fm                   (3 KB)
                kv_fm = res1.tile([P, KE, NKV], BF16, tag="tagA", name="kv_fm")
                qln_fm = res1.tile([P, KE, NQ], BF16, tag="tagD",
                                   name="qln_fm")

                # ---------------- stage A: LN + transpose ----------------
                def ln_rows(xrows, gain, bias_, n_rows, fm_dst, tm_dst,
                            apply_gb=True):
                    with tc.tile_pool(name="lnx", bufs=2) as xp, \
                         tc.tile_pool(name="lnst", bufs=4) as stp, \
                         tc.tile_pool(name="lnps", bufs=2, space="PSUM") as psp, \
                         tc.tile_pool(name="lng", bufs=1) as gp:
                        gt = gp.tile([P, E], BF16, name="ln_g")
                        bt = gp.tile([P, E], BF16, name="ln_b")
                        nc.gpsimd.dma_start(out=gt, in_=_bcast_ap(gain, P))
                        nc.gpsimd.dma_start(out=bt, in_=_bcast_ap(bias_, P))
                        for it in range(_ceil(n_rows, P)):
                            r0 = it * P
                            p = min(P, n_rows - r0)
                            xt = xp.tile([P, E], F32, name="ln_x")
                            nc.sync.dma_start(out=xt[:p],
                                              in_=xrows[r0:r0 + p, :])
                            x3 = xt.rearrange("p (n f) -> p n f", n=2)
                            st = stp.tile([P, 2, 6], F32, name="ln_st")
                            for i in range(2):
                                nc.vector.bn_stats(out=st[:p, i, :],
                                                   in_=x3[:p, i, :])
                            mv = stp.tile([P, 2], F32, name="ln_mv")
                            nc.vector.bn_aggr(out=mv[:p], in_=st[:p])
                            rs = stp.tile([P, 1], F32, name="ln_rs")
                            nc.scalar.activation(out=rs[:p], in_=mv[:p, 1:2],
                                                 func=AF.Sqrt, bias=epst[:p])
                            nc.vector.reciprocal(out=rs[:p], in_=rs[:p])
                            y = xp.tile([P, E], BF16, name="ln_y")
                            nc.vector.tensor_scalar(
                                out=y[:p], in0=xt[:p], scalar1=mv[:p, 0:1],
                                scalar2=rs[:p], op0=ALU.subtract,
                                op1=ALU.mult)
                            if apply_gb:
                                nc.vector.tensor_mul(y[:p], y[:p], gt[:p])
                                nc.vector.tensor_add(y[:p], y[:p], bt[:p])
                            if tm_dst is not None:
                                nc.sync.dma_start(out=tm_dst[r0:r0 + p, :],
                                                  in_=y[:p])
                            tp = psp.tile([P, KE, P], BF16, name="ln_tp")
                            for e in range(KE):
                                nc.tensor.transpose(
                                    out=tp[:, e, :p],
                                    in_=y[:p, e * P:(e + 1) * P],
                                    identity=ident[:p, :p])
                            nc.scalar.copy(out=fm_dst[:, :, r0:r0 + p],
                                           in_=tp[:, :, :p])

                ln_rows(x[:NKV, :], gkv, bbkv, NKV, kv_fm, kv_tm)
                ln_rows(x[NKV:, :], gq, bbq, NQ, qln_fm, None,
                        apply_gb=False)

                # ---------------- stage B: QKV ----------------
                k_fm = res1.tile([P, KE, NKV], BF16, tag="tagB", name="k_fm")
                v_res = res1.tile([P, T, 2, E], BF16, tag="tagC",
                                  name="v_res")
                q_fm = res1.tile([P, KE, NQ], BF16, tag="tagE", name="q_fm")

                def fm_gemm(w_src, bias_h, src_fm, n_total, dst_fm, act,
                            mo, kc, tag):
                    with tc.tile_pool(name=tag + "w", bufs=1) as wp, \
                         tc.tile_pool(name=tag + "b", bufs=1) as bp, \
                         tc.tile_pool(name=tag + "ps", bufs=4,
                                      space="PSUM") as pp:
                        if not isinstance(w_src, bass.DRamTensorHandle):
                            wsb = w_src
                        else:
                            wsb = wp.tile([P, kc, mo * P], BF16,
                                          name=tag + "_w")
                            for k in range(kc):
                                nc.sync.dma_start(
                                    out=wsb[:, k, :],
                                    in_=w_src[k * P:(k + 1) * P, :])
                        bsb = bp.tile([P, mo], F32, name=tag + "_b")
                        nc.sync.dma_start(out=bsb, in_=_col_ap(bias_h, mo))
                        for n in range(_ceil(n_total, NT)):
                            n0 = n * NT
                            w = min(NT, n_total - n0)
                            for m in range(mo):
                                ps = pp.tile([P, NT], F32, name=tag + "_ps")
                                for k in range(kc):
                                    nc.tensor.matmul(
                                        ps[:, :w],
                                        lhsT=wsb[:, k, m * P:(m + 1) * P],
                                        rhs=src_fm[:, k, n0:n0 + w],
                                        start=(k == 0), stop=(k == kc - 1))
                                nc.scalar.activation(
                                    out=dst_fm[:, m, n0:n0 + w],
                                    in_=ps[:, :w], func=act,
                                    bias=bsb[:, m:m + 1])

                fm_gemm(wkT, bk, kv_fm, NKV, k_fm, AF.Identity, KE, KE, "bk")
                fm_gemm(wqT, bqs, qln_fm, NQ, q_fm, AF.Identity, KE, KE,
                        "bq")

                # v: token-major per (t, half) via swapped orientation
                with tc.tile_pool(name="vw", bufs=1) as vwp, \
                     tc.tile_pool(name="vb", bufs=1) as vbp, \
                     tc.tile_pool(name="vps", bufs=3, space="PSUM") as vpp:
                    wsb = vwp.tile([P, KE, E], BF16, name="v_w")
                    for k in range(KE):
                        nc.sync.dma_start(out=wsb[:, k, :],
                                          in_=wvT[k * P:(k + 1) * P, :])
                    bvb = vbp.tile([P, E], F32, name="v_b")
                    nc.sync.dma_start(out=bvb, in_=_bcast_ap(bv, P))
                    for t_ in range(T):
                        for half in range(2):
                            r0 = t_ * NF + half * P
                            pr = P if half == 0 else NF - P
                            ps = vpp.tile([P, E], F32, name="v_ps")
                            for j in range(2):
                                for k in range(KE):
                                    nc.tensor.matmul(
                                        ps[:pr, j * EH:(j + 1) * EH],
                                        lhsT=kv_fm[:, k, r0:r0 + pr],
                                        rhs=wsb[:, k, j * EH:(j + 1) * EH],
                                        start=(k == 0), stop=(k == KE - 1))
                            nc.vector.tensor_add(v_res[:pr, t_, half, :],
                                                 ps[:pr, :], bvb[:pr, :])

                # ---------------- stage C: attention ----------------
                o_fm = res1.tile([P, KE, NKV], BF16, tag="tagA", name="o_fm")

                nch = [(0, P), (P, NQ - P)]
                tpairs = [(t0, min(2, T - t0)) for t0 in range(0, T, 2)]
                with tc.tile_pool(name="cat", bufs=3) as cat, \
                     tc.tile_pool(name="cst", bufs=4) as cst, \
                     tc.tile_pool(name="cps", bufs=2, space="PSUM") as cps, \
                     tc.tile_pool(name="cpt", bufs=2, space="PSUM") as cpt, \
                     tc.tile_pool(name="cpo", bufs=2, space="PSUM") as cpo:
                    for hp in range(H // 2):
                        for (tb, tn) in tpairs:
                            po = cpo.tile([P, 2, NQ], F32, name="c_po")
                            for pi in range(2):
                                d0 = pi * D
                                asbs = []
                                rc = cst.tile([P, 2, 2], F32, name="c_rc")
                                sm = cst.tile([P, 2, 2], F32, name="c_sm")
                                for j, (n0, pn) in enumerate(nch):
                                    ps = cps.tile([P, 2 * NQ], F32,
                                                  name=f"c_ps{j}")
                                    nc.tensor.matmul(
                                        ps[:pn, :tn * NQ],
                                        lhsT=q_fm[d0:d0 + D, hp, n0:n0 + pn],
                                        rhs=k_fm[d0:d0 + D, hp,
                                                 tb * NF:(tb + tn) * NF],
                                        start=True, stop=True)
                                    asb = cat.tile([P, 2, NQ], BF16,
                                                   name=f"c_asb{j}")
                                    nc.scalar.activation(
                                        out=asb[:pn, :tn, :].rearrange(
                                            "p t n -> p (t n)"),
                                        in_=ps[:pn, :tn * NQ], func=AF.Exp)
                                    nc.vector.reduce_sum(
                                        out=sm[:pn, j, :tn],
                                        in_=asb[:pn, :tn, :], axis=AX.X)
                                    asbs.append(asb)
                                nc.vector.reciprocal(out=rc, in_=sm)
                                for j, (n0, pn) in enumerate(nch):
                                    for ti in range(tn):
                                        nc.vector.tensor_scalar_mul(
                                            asbs[j][:pn, ti, :],
                                            in0=asbs[j][:pn, ti, :],
                                            scalar1=rc[:pn, j, ti:ti + 1])
                                pt = cpt.tile([P, 2, 2, NQ], BF16,
                                              name="c_pt")
                                for ti in range(tn):
                                    for jn, (n0, pn) in enumerate(nch):
                                        for jm, (m0, mj) in enumerate(nch):
                                            nc.tensor.transpose(
                                                out=pt[:mj, ti, jm,
                                                       n0:n0 + pn],
                                                in_=asbs[jn][:pn, ti,
                                                             m0:m0 + mj],
                                                identity=ident[:pn, :pn])
                                atT = cat.tile([P, 2, 2, NQ], BF16,
                                               name="c_atT")
                                nc.scalar.copy(out=atT[:, :tn],
                                               in_=pt[:, :tn])
                                for ti in range(tn):
                                    for jm, (m0, mj) in enumerate(nch):
                                        nc.tensor.matmul(
                                            po[d0:d0 + D, ti, :],
                                            lhsT=v_res[:mj, tb + ti, jm,
                                                       hp * P + d0:
                                                       hp * P + d0 + D],
                                            rhs=atT[:mj, ti, jm, :],
                                            start=(jm == 0), stop=(jm == 1))
                            nc.scalar.copy(
                                out=o_fm[:, hp, tb * NF:(tb + tn) * NF],
                                in_=po[:, :tn, :])

                # ---------------- stage D: out_proj ----------------
                att_fm = res1.tile([P, KE, NKV], BF16, tag="tagB",
                                   name="att_fm")
                wo_sb = res1.tile([P, KE, E], BF16, tag="tagD", name="wo_sb")
                with tc.tile_pool(name="dob", bufs=1) as dbp, \
                     tc.tile_pool(name="dops", bufs=4, space="PSUM") as dpp:
                    for k in range(KE):
                        nc.sync.dma_start(out=wo_sb[:, k, :],
                                          in_=woT[k * P:(k + 1) * P, :])
                    bsb = dbp.tile([P, KE], F32, name="do_b")
                    nc.sync.dma_start(out=bsb, in_=_col_ap(bo, KE))
                    for n in range(NTILES):
                        n0 = n * NT
                        for m in range(KE):
                            ps = dpp.tile([P, NT], F32, name="do_ps")
                            for k in range(KE):
                                nc.tensor.matmul(
                                    ps[:, :],
                                    lhsT=wo_sb[:, k, m * P:(m + 1) * P],
                                    rhs=o_fm[:, k, n0:n0 + NT],
                                    start=(k == 0), stop=(k == KE - 1))
                            nc.scalar.activation(
                                out=att_fm[:, m, n0:n0 + NT], in_=ps[:, :],
                                func=AF.Identity, bias=bsb[:, m:m + 1])

                # ---------------- stage E1: mlpq GEMM1 ----------------
                def mlp_g1(wT_h, bias_h, src_fm, dst_nt, wpool, wtag,
                           stgpool, stgtag, tag):
                    with tc.tile_pool(name=tag + "b", bufs=1) as bp, \
                         tc.tile_pool(name=tag + "ps", bufs=4,
                                      space="PSUM") as pp:
                        bsb = bp.tile([P, KM], F32, name=tag + "_b")
                        nc.sync.dma_start(out=bsb, in_=_col_ap(bias_h, KM))
                        for mh in range(2):
                            wsb = wpool.tile([P, KE, MH], BF16, tag=wtag,
                                             name=tag + f"_w{mh}")
                            for k in range(KE):
                                nc.sync.dma_start(
                                    out=wsb[:, k, :],
                                    in_=wT_h[k * P:(k + 1) * P,
                                             mh * MH:(mh + 1) * MH])
                            for n in range(NTILES):
                                n0 = n * NT
                                for q in range(2):
                                    stg = stgpool.tile([P, 8, NT], BF16,
                                                       tag=stgtag, bufs=2,
                                                       name=tag + "_stg")
                                    for m8 in range(8):
                                        mi = mh * 16 + q * 8 + m8
                                        ps = pp.tile([P, NT], F32,
                                                     name=tag + "_ps")
                                        for k in range(KE):
                                            nc.tensor.matmul(
                                                ps[:, :],
                                                lhsT=wsb[:, k,
                                                         (q * 8 + m8) * P:
                                                         (q * 8 + m8 + 1) * P],
                                                rhs=src_fm[:, k,
                                                           n0:n0 + NT],
                                                start=(k == 0),
                                                stop=(k == KE - 1))
                                        nc.scalar.activation(
                                            out=stg[:, m8, :], in_=ps[:, :],
                                            func=AF.Gelu,
                                            bias=bsb[:, mi:mi + 1])
                                    nc.sync.dma_start(
                                        out=dst_nt[n][:, mh * 16 + q * 8:
                                                      mh * 16 + q * 8 + 8,
                                                      :],
                                        in_=stg)

                mlp_g1(w1qT, b1q, att_fm, h1q_nt, res1, "tagC", res1, "tagE1S", "e1")

            # =================== res2: stages E2..G2 ===================
            with tc.tile_pool(name="res2", bufs=1) as res2:
                # tagH: ln2t_fm (47 KB); tagW2: w2q -> w2 (64 KB)
                # tagW1: w1 halves (32 KB); quarters via scoped pools
                ln2t_fm = res2.tile([P, KE, NKV], BF16, tag="tagH",
                                    name="ln2t_fm")

                def ln_tm(stp, src, dst, gg, bb, p, apply_gb=True):
                    x3 = src.rearrange("p (n f) -> p n f", n=2)
                    st = stp.tile([P, 2, 6], F32, name="f_st")
                    for i in range(2):
                        nc.vector.bn_stats(out=st[:p, i, :], in_=x3[:p, i, :])
                    mv = stp.tile([P, 2], F32, name="f_mv")
                    nc.vector.bn_aggr(out=mv[:p], in_=st[:p])
                    rs = stp.tile([P, 1], F32, name="f_rs")
                    nc.scalar.activation(out=rs[:p], in_=mv[:p, 1:2],
                                         func=AF.Sqrt, bias=epst[:p])
                    nc.vector.reciprocal(out=rs[:p], in_=rs[:p])
                    nc.vector.tensor_scalar(
                        out=dst[:p], in0=src[:p], scalar1=mv[:p, 0:1],
                        scalar2=rs[:p], op0=ALU.subtract, op1=ALU.mult)
                    if apply_gb:
                        nc.vector.tensor_mul(dst[:p], dst[:p], gg[:p])
                        nc.vector.tensor_add(dst[:p], dst[:p], bb[:p])

                def mlp_g2(w_sb, src_nt, tag, epilogue, hq_first=None):
                    with tc.tile_pool(name=tag + "ps", bufs=2,
                                      space="PSUM") as pp:
                        for n in range(NTILES):
                            if n == 0 and hq_first is not None:
                                hqa, hqb = hq_first
                            else:
                                hqa = res2.tile([P, KM // 2, NT], BF16,
                                                tag="tagW1",
                                                name=tag + "_ha")
                                hqb = res2.tile([P, KM // 2, NT], BF16,
                                                tag="tagW1B",
                                                name=tag + "_hb")
                                for qtr in range(2):
                                    nc.sync.dma_start(
                                        out=hqa[:, qtr * 8:(qtr + 1) * 8,
                                                :],
                                        in_=src_nt[n][:,
                                                      qtr * 8:(qtr + 1) * 8,
                                                      :])
                                    nc.sync.dma_start(
                                        out=hqb[:, qtr * 8:(qtr + 1) * 8,
                                                :],
                                        in_=src_nt[n][:,
                                                      16 + qtr * 8:
                                                      16 + (qtr + 1) * 8,
                                                      :])
                            subs = [(0, P), (P, P), (2 * P, P),
                                    (3 * P, NT - 3 * P)]
                            for (s0, pr) in subs:
                                ps = pp.tile([P, E], F32, name=tag + "_ps")
                                for j in range(2):
                                    for k in range(KM):
                                        hk = hqa if k < 16 else hqb
                                        nc.tensor.matmul(
                                            ps[:pr, j * EH:(j + 1) * EH],
                                            lhsT=hk[:, k % 16, s0:s0 + pr],
                                            rhs=w_sb[:, k,
                                                     j * EH:(j + 1) * EH],
                                            start=(k == 0),
                                            stop=(k == KM - 1))
                                epilogue(ps, n * NT + s0, pr)

                # ---------------- stage E2 + F ----------------
                # load the first GEMM2 input tile before the 8 MB weight so
                # the first matmuls only wait for the leading weight chunks
                hq0a = res2.tile([P, KM // 2, NT], BF16, tag="tagW1",
                                 name="e2_h0a")
                hq0b = res2.tile([P, KM // 2, NT], BF16, tag="tagW1B",
                                 name="e2_h0b")
                for qtr in range(2):
                    nc.sync.dma_start(
                        out=hq0a[:, qtr * 8:(qtr + 1) * 8, :],
                        in_=h1q_nt[0][:, qtr * 8:(qtr + 1) * 8, :])
                    nc.sync.dma_start(
                        out=hq0b[:, qtr * 8:(qtr + 1) * 8, :],
                        in_=h1q_nt[0][:, 16 + qtr * 8:16 + (qtr + 1) * 8, :])
                w2q_sb = res2.tile([P, KM, E], BF16, tag="tagW2",
                                   name="w2q_sb")
                for k in range(KM):
                    nc.sync.dma_start(out=w2q_sb[:, k, :],
                                      in_=w2qT[k * P:(k + 1) * P, :])
                with tc.tile_pool(name="fg", bufs=1) as fg, \
                     tc.tile_pool(name="fst", bufs=4) as fst, \
                     tc.tile_pool(name="fw", bufs=2) as fwp, \
                     tc.tile_pool(name="fkv", bufs=2) as fkv, \
                     tc.tile_pool(name="ftp", bufs=2, space="PSUM") as ftp:
                    b2qb = fg.tile([P, E], BF16, name="f_b2q")
                    g1t = fg.tile([P, E], BF16, name="f_g1")
                    b1t = fg.tile([P, E], BF16, name="f_b1")
                    nc.gpsimd.dma_start(out=b2qb, in_=_bcast_ap(b2q, P))
                    nc.gpsimd.dma_start(out=g1t, in_=_bcast_ap(gres, P))
                    nc.gpsimd.dma_start(out=b1t, in_=_bcast_ap(bbres, P))

                    def e2_epilogue(ps, r0, pr):
                        kvt = fkv.tile([P, E], BF16, name="f_kv")
                        nc.sync.dma_start(out=kvt[:pr],
                                          in_=kv_tm[r0:r0 + pr, :])
                        qs = fwp.tile([P, E], F32, name="f_qs")
                        nc.vector.tensor_add(qs[:pr], ps[:pr, :], b2qb[:pr])
                        nc.vector.tensor_add(qs[:pr], qs[:pr], kvt[:pr])
                        y1 = fwp.tile([P, E], F32, name="f_y1")
                        ln_tm(fst, qs, y1, g1t, b1t, pr)
                        y2 = fwp.tile([P, E], BF16, name="f_y2")
                        ln_tm(fst, y1, y2, None, None, pr, apply_gb=False)
                        tp = ftp.tile([P, KE, P], BF16, name="f_tp")
                        for e in range(KE):
                            nc.tensor.transpose(
                                out=tp[:, e, :pr],
                                in_=y2[:pr, e * P:(e + 1) * P],
                                identity=ident[:pr, :pr])
                        nc.scalar.copy(out=ln2t_fm[:, :, r0:r0 + pr],
                                       in_=tp[:, :, :pr])

                    mlp_g2(w2q_sb, h1q_nt, "e2", e2_epilogue,
                           hq_first=(hq0a, hq0b))

                # ---------------- stage G1 ----------------
                mlp_g1(w1T, b1, ln2t_fm, h1_nt, res2, "tagW1", res2, "tagG1S", "g1")

                # ---------------- stage G2 ----------------
                w2_sb = res2.tile([P, KM, E], BF16, tag="tagW2", name="w2_sb")
                for k in range(KM):
                    nc.sync.dma_start(out=w2_sb[:, k, :],
                                      in_=w2T[k * P:(k + 1) * P, :])
                with tc.tile_pool(name="gg", bufs=1) as ggp, \
                     tc.tile_pool(name="gout", bufs=2) as gop:
                    b2b = ggp.tile([P, E], F32, name="g_b2")
                    nc.sync.dma_start(out=b2b, in_=_bcast_ap(b2, P))

                    def g2_epilogue(ps, r0, pr):
                        ot = gop.tile([P, E], F32, name="g_out")
                        nc.vector.tensor_add(ot[:pr], ps[:pr, :], b2b[:pr])
                        nc.sync.dma_start(out=out[r0:r0 + pr, :],
                                          in_=ot[:pr])

                    mlp_g2(w2_sb, h1_nt, "g2", g2_epilogue)

    nc.compile()
    return nc


_NC = None


def _get_nc():
    global _NC
    if _NC is None:
        _NC = build_nc()
    return _NC


def _prep_in_maps(inputs):
    f32 = lambda a: np.ascontiguousarray(np.asarray(a, dtype=np.float32))
    bf = lambda a: np.ascontiguousarray(
        np.asarray(a, dtype=np.float32).astype(ml_dtypes.bfloat16))
    x = f32(inputs["inputs"])                       # (B,HW,NF,E)
    ipw = f32(inputs["in_proj_w"])
    ipb = f32(inputs["in_proj_b"])
    wq, wk, wv = ipw[:E], ipw[E:2 * E], ipw[2 * E:]
    bq, bk_, bv_ = ipb[:E], ipb[E:2 * E], ipb[2 * E:]
    s = 1.0 / np.sqrt(np.float32(D))
    # fold q-LN gain/bias into wq/bq (q_ln feeds only the q projection)
    gq_v = f32(inputs["ln_q_g"])
    bq_v = f32(inputs["ln_q_b"])
    wq_f = wq * gq_v[None, :]
    bq_f = bq + wq @ bq_v
    # fold ln2 gain/bias into mlp_w1/b1 (ln2 feeds only the final MLP)
    g2_v = f32(inputs["ln2_g"])
    b2_v = f32(inputs["ln2_b"])
    w1_f = f32(inputs["mlp_w1"]) * g2_v[None, :]
    b1_f = f32(inputs["mlp_b1"]) + f32(inputs["mlp_w1"]) @ b2_v
    shared = {
        "wqT": bf(wq_f.T * s), "wkT": bf(wk.T), "wvT": bf(wv.T),
        "woT": bf(f32(inputs["out_proj_w"]).T),
        "w1qT": bf(f32(inputs["mlpq_w1"]).T),
        "w2qT": bf(f32(inputs["mlpq_w2"]).T),
        "w1T": bf(w1_f.T),
        "w2T": bf(f32(inputs["mlp_w2"]).T),
        "bqs": f32(bq_f * s), "bk": f32(bk_), "bv": f32(bv_),
        "bo": f32(inputs["out_proj_b"]),
        "b1q": f32(inputs["mlpq_b1"]), "b2q": f32(inputs["mlpq_b2"]),
        "b1": f32(b1_f), "b2": f32(inputs["mlp_b2"]),
        "gq": f32(inputs["ln_q_g"]), "bbq": f32(inputs["ln_q_b"]),
        "gkv": f32(inputs["ln_kv_g"]), "bbkv": f32(inputs["ln_kv_b"]),
        "gres": f32(inputs["res_ln_g"]), "bbres": f32(inputs["res_ln_b"]),
        "gln2": f32(inputs["ln2_g"]), "bbln2": f32(inputs["ln2_b"]),
    }
    return [dict(shared, x=np.ascontiguousarray(x[b].reshape(NTOK, E)))
            for b in range(B)]


def _run(inputs, trace=False):
    from concourse.bass_utils import run_bass_kernel_spmd
    nc = _get_nc()
    in_maps = _prep_in_maps(inputs)
    res = run_bass_kernel_spmd(nc, in_maps, core_ids=list(range(B)),
                               trace=trace)
    outs = np.stack([r["out"].reshape(T, NF, E) for r in res.results])
    return outs, res


def kernel(**inputs) -> np.ndarray:
    outs, _ = _run(inputs, trace=False)
    return outs


# revision 6
# speedup vs baseline: 1.0534x; 1.0534x over previous
"""Trainium2 Bass kernel for LocalBackwardTemporalAttention (optimized v2).

Data-parallel over batch B=8 (one element per core, no collectives).
Single fused on-chip pipeline; SBUF-resident intermediates with
tag-chained slot reuse (two sequential resident pools res1/res2):

  A: LN(x) -> kv_fm (SBUF fm bf16), kv_tm -> DRAM; q_ln -> SBUF
  B: QKV GEMMs from kv_fm -> k_fm, v_res (per-t token-major), q_fm
  C: attention per (hp, t-pair, pi): S = q@k^T (no max-sub) -> exp ->
     sum/recip/scale -> PE-transpose -> attnT@v -> o_fm (SBUF)
  D: out_proj o_fm -> att_fm (SBUF)
  E1: mlpq GEMM1 (+gelu), M-halved weights -> h1q (DRAM)
  --- res1 -> res2 ---
  E2+F: per 128-token chunk: GEMM2 swapped + b2q + kv_tm residual
        -> res_ln -> ln2 -> PE-transpose -> ln2t_fm (SBUF)
  G1: mlp GEMM1 (+gelu), M-halved -> h1 (DRAM)
  G2: GEMM2 swapped + b2 -> out (DRAM, token-major f32)

All GEMMs bf16 in / f32 PSUM accum. DRAM round trips only for kv_tm,
h1q, h1. DMA transfers are 0.125-4 MB (vs ~1100 tiny DMAs in v1).
"""

import sys

sys.path.insert(0, "/opt/trn_rl_repo")

import numpy as np
import ml_dtypes

import concourse.bass as bass
import concourse.bacc as bacc_mod
import concourse.mybir as mybir
import concourse.tile as tile
from concourse.masks import make_identity

F32 = mybir.dt.float32
BF16 = mybir.dt.bfloat16
AF = mybir.ActivationFunctionType
ALU = mybir.AluOpType
AX = mybir.AxisListType

B, HW, NF, E, M, H = 8, 16, 196, 1024, 4096, 16
T, D = HW - 1, E // H            # 15, 64
NKV, NQ, NTOK = T * NF, NF, HW * NF
EPS = 1e-6
P = 128
NT = 490                         # fm GEMM token tile (2940 = 6*490)
KE, KM = E // P, M // P          # 8, 32
EH = E // 2                      # 512
NTILES = NKV // NT               # 6
MH = M // 2                      # 2048


def _ceil(a, b):
    return -(-a // b)


def _bcast_ap(handle, n):
    a = handle[:]
    return bass.AP(tensor=a.tensor, offset=a.offset, ap=[[0, n], list(a.ap[0])])


def _col_ap(handle, mo):
    a = handle[:]
    return bass.AP(tensor=a.tensor, offset=a.offset, ap=[[1, P], [P, mo]])


def build_nc():
    nc = bacc_mod.Bacc(None, target_bir_lowering=False)
    t = lambda n, s, d: nc.dram_tensor(n, s, d, kind="ExternalInput")

    x = t("x", [NTOK, E], F32)
    wqT = t("wqT", [E, E], BF16)
    wkT = t("wkT", [E, E], BF16)
    wvT = t("wvT", [E, E], BF16)
    woT = t("woT", [E, E], BF16)
    w1qT = t("w1qT", [E, M], BF16)
    w2qT = t("w2qT", [M, E], BF16)
    w1T = t("w1T", [E, M], BF16)
    w2T = t("w2T", [M, E], BF16)
    bqs = t("bqs", [E], F32)
    bk = t("bk", [E], F32)
    bv = t("bv", [E], F32)
    bo = t("bo", [E], F32)
    b1q = t("b1q", [M], F32)
    b2q = t("b2q", [E], F32)
    b1 = t("b1", [M], F32)
    b2 = t("b2", [E], F32)
    gq = t("gq", [E], F32)
    bbq = t("bbq", [E], F32)
    gkv = t("gkv", [E], F32)
    bbkv = t("bbkv", [E], F32)
    gres = t("gres", [E], F32)
    bbres = t("bbres", [E], F32)
    gln2 = t("gln2", [E], F32)
    bbln2 = t("bbln2", [E], F32)
    out = nc.dram_tensor("out", [NKV, E], F32, kind="ExternalOutput")

    with tile.TileContext(nc) as tc:
        with tc.tile_pool(name="dram", bufs=1, space="DRAM") as dram, \
             tc.tile_pool(name="consts", bufs=1) as consts:
            kv_tm = dram.tile([NKV, E], BF16)
            h1q_nt = dram.tile([NTILES, P, KM, NT], BF16)
            h1_nt = dram.tile([NTILES, P, KM, NT], BF16)

            ident = consts.tile([P, P], BF16)
            make_identity(nc, ident)
            epst = consts.tile([P, 1], F32)
            nc.vector.memset(epst, EPS)

            # =================== res1: stages A..E1 ===================
            with tc.tile_pool(name="res1", bufs=1) as res1:
                # tagA: kv_fm -> o_fm          (47 KB/part)
                # tagB: k_fm -> att_fm         (47 KB)
                # tagC: v_res -> w1q halves    (60 KB)
                # tagD: qln_fm -> wo           (16 KB)
                # tagE: q_fm                   (3 KB)
                kv_fm = res1.tile([P, KE, NKV], BF16, tag="tagA", name="kv_fm")
                qln_fm = res1.tile([P, KE, NQ], BF16, tag="tagD",
                                   name="qln_fm")

                # ---------------- stage A: LN + transpose ----------------
                def ln_rows(xrows, gain, bias_, n_rows, fm_dst, tm_dst,
                            apply_gb=True):
                    with tc.tile_pool(name="lnx", bufs=2) as xp, \
                         tc.tile_pool(name="lnst", bufs=4) as stp, \
                         tc.tile_pool(name="lnps", bufs=2, space="PSUM") as psp, \
                         tc.tile_pool(name="lng", bufs=1) as gp:
                        if apply_gb:
                            gt = gp.tile([P, E], BF16, name="ln_g")
                            bt = gp.tile([P, E], BF16, name="ln_b")
                            nc.gpsimd.dma_start(out=gt,
                                                in_=_bcast_ap(gain, P))
                            nc.gpsimd.dma_start(out=bt,
                                                in_=_bcast_ap(bias_, P))
                        for it in range(_ceil(n_rows, P)):
                            r0 = it * P
                            p = min(P, n_rows - r0)
                            xt = xp.tile([P, E], F32, name="ln_x")
                            nc.sync.dma_start(out=xt[:p],
                                              in_=xrows[r0:r0 + p, :])
                            x3 = xt.rearrange("p (n f) -> p n f", n=2)
                            st = stp.tile([P, 2, 6], F32, name="ln_st")
                            for i in range(2):
                                nc.vector.bn_stats(out=st[:p, i, :],
                                                   in_=x3[:p, i, :])
                            mv = stp.tile([P, 2], F32, name="ln_mv")
                            nc.vector.bn_aggr(out=mv[:p], in_=st[:p])
                            rs = stp.tile([P, 1], F32, name="ln_rs")
                            nc.scalar.activation(out=rs[:p], in_=mv[:p, 1:2],
                                                 func=AF.Sqrt, bias=epst[:p])
                            nc.vector.reciprocal(out=rs[:p], in_=rs[:p])
                            y = xp.tile([P, E], BF16, name="ln_y")
                            nc.vector.tensor_scalar(
                                out=y[:p], in0=xt[:p], scalar1=mv[:p, 0:1],
                                scalar2=rs[:p], op0=ALU.subtract,
                                op1=ALU.mult)
                            if apply_gb:
                                nc.vector.tensor_mul(y[:p], y[:p], gt[:p])
                                nc.vector.tensor_add(y[:p], y[:p], bt[:p])
                            if tm_dst is not None:
                                nc.sync.dma_start(out=tm_dst[r0:r0 + p, :],
                                                  in_=y[:p])
                            tp = psp.tile([P, KE, P], BF16, name="ln_tp")
                            for e in range(KE):
                                nc.tensor.transpose(
                                    out=tp[:, e, :p],
                                    in_=y[:p, e * P:(e + 1) * P],
                                    identity=ident[:p, :p])
                            nc.scalar.copy(out=fm_dst[:, :, r0:r0 + p],
                                           in_=tp[:, :, :p])

                ln_rows(x[:NKV, :], gkv, bbkv, NKV, kv_fm, kv_tm,
                        apply_gb=False)
                ln_rows(x[NKV:, :], gq, bbq, NQ, qln_fm, None,
                        apply_gb=False)

                # ---------------- stage B: QKV ----------------
                k_fm = res1.tile([P, KE, NKV], BF16, tag="tagB", name="k_fm")
                v_res = res1.tile([P, T, 2, E], BF16, tag="tagC",
                                  name="v_res")
                q_fm = res1.tile([P, KE, NQ], BF16, tag="tagE", name="q_fm")

                def fm_gemm(w_src, bias_h, src_fm, n_total, dst_fm, act,
                            mo, kc, tag):
                    with tc.tile_pool(name=tag + "w", bufs=1) as wp, \
                         tc.tile_pool(name=tag + "b", bufs=1) as bp, \
                         tc.tile_pool(name=tag + "ps", bufs=4,
                                      space="PSUM") as pp:
                        if not isinstance(w_src, bass.DRamTensorHandle):
                            wsb = w_src
                        else:
                            wsb = wp.tile([P, kc, mo * P], BF16,
                                          name=tag + "_w")
                            for k in range(kc):
                                nc.sync.dma_start(
                                    out=wsb[:, k, :],
                                    in_=w_src[k * P:(k + 1) * P, :])
                        bsb = bp.tile([P, mo], F32, name=tag + "_b")
                        nc.sync.dma_start(out=bsb, in_=_col_ap(bias_h, mo))
                        for n in range(_ceil(n_total, NT)):
                            n0 = n * NT
                            w = min(NT, n_total - n0)
                            for m in range(mo):
                                ps = pp.tile([P, NT], F32, name=tag + "_ps")
                                for k in range(kc):
                                    nc.tensor.matmul(
                                        ps[:, :w],
                                        lhsT=wsb[:, k, m * P:(m + 1) * P],
                                        rhs=src_fm[:, k, n0:n0 + w],
                                        start=(k == 0), stop=(k == kc - 1))
                                nc.scalar.activation(
                                    out=dst_fm[:, m, n0:n0 + w],
                                    in_=ps[:, :w], func=act,
                                    bias=bsb[:, m:m + 1])

                fm_gemm(wkT, bk, kv_fm, NKV, k_fm, AF.Identity, KE, KE, "bk")
                fm_gemm(wqT, bqs, qln_fm, NQ, q_fm, AF.Identity, KE, KE,
                        "bq")

                # v: token-major per (t, half) via swapped orientation
                with tc.tile_pool(name="vw", bufs=1) as vwp, \
                     tc.tile_pool(name="vb", bufs=1) as vbp, \
                     tc.tile_pool(name="vps", bufs=3, space="PSUM") as vpp:
                    wsb = vwp.tile([P, KE, E], BF16, name="v_w")
                    for k in range(KE):
                        nc.sync.dma_start(out=wsb[:, k, :],
                                          in_=wvT[k * P:(k + 1) * P, :])
                    bvb = vbp.tile([P, E], F32, name="v_b")
                    nc.sync.dma_start(out=bvb, in_=_bcast_ap(bv, P))
                    for t_ in range(T):
                        for half in range(2):
                            r0 = t_ * NF + half * P
                            pr = P if half == 0 else NF - P
                            ps = vpp.tile([P, E], F32, name="v_ps")
                            for j in range(2):
                                for k in range(KE):
                                    nc.tensor.matmul(
                                        ps[:pr, j * EH:(j + 1) * EH],
                                        lhsT=kv_fm[:, k, r0:r0 + pr],
                                        rhs=wsb[:, k, j * EH:(j + 1) * EH],
                                        start=(k == 0), stop=(k == KE - 1))
                            nc.vector.tensor_add(v_res[:pr, t_, half, :],
                                                 ps[:pr, :], bvb[:pr, :])

                # ---------------- stage C: attention ----------------
                o_fm = res1.tile([P, KE, NKV], BF16, tag="tagA", name="o_fm")

                nch = [(0, P), (P, NQ - P)]
                tpairs = [(t0, min(2, T - t0)) for t0 in range(0, T, 2)]
                with tc.tile_pool(name="cat", bufs=3) as cat, \
                     tc.tile_pool(name="cst", bufs=4) as cst, \
                     tc.tile_pool(name="cps", bufs=2, space="PSUM") as cps, \
                     tc.tile_pool(name="cpt", bufs=2, space="PSUM") as cpt, \
                     tc.tile_pool(name="cpo", bufs=2, space="PSUM") as cpo:
                    for hp in range(H // 2):
                        for (tb, tn) in tpairs:
                            po = cpo.tile([P, 2, NQ], F32, name="c_po")
                            for pi in range(2):
                                d0 = pi * D
                                asbs = []
                                rc = cst.tile([P, 2, 2], F32, name="c_rc")
                                sm = cst.tile([P, 2, 2], F32, name="c_sm")
                                for j, (n0, pn) in enumerate(nch):
                                    ps = cps.tile([P, 2 * NQ], F32,
                                                  name=f"c_ps{j}")
                                    nc.tensor.matmul(
                                        ps[:pn, :tn * NQ],
                                        lhsT=q_fm[d0:d0 + D, hp, n0:n0 + pn],
                                        rhs=k_fm[d0:d0 + D, hp,
                                                 tb * NF:(tb + tn) * NF],
                                        start=True, stop=True)
                                    asb = cat.tile([P, 2, NQ], BF16,
                                                   name=f"c_asb{j}")
                                    nc.scalar.activation(
                                        out=asb[:pn, :tn, :].rearrange(
                                            "p t n -> p (t n)"),
                                        in_=ps[:pn, :tn * NQ], func=AF.Exp)
                                    nc.vector.reduce_sum(
                                        out=sm[:pn, j, :tn],
                                        in_=asb[:pn, :tn, :], axis=AX.X)
                                    asbs.append(asb)
                                nc.vector.reciprocal(out=rc, in_=sm)
                                for j, (n0, pn) in enumerate(nch):
                                    for ti in range(tn):
                                        nc.vector.tensor_scalar_mul(
                                            asbs[j][:pn, ti, :],
                                            in0=asbs[j][:pn, ti, :],
                                            scalar1=rc[:pn, j, ti:ti + 1])
                                pt = cpt.tile([P, 2, 2, NQ], BF16,
                                              name="c_pt")
                                for ti in range(tn):
                                    for jn, (n0, pn) in enumerate(nch):
                                        for jm, (m0, mj) in enumerate(nch):
                                            nc.tensor.transpose(
                                                out=pt[:mj, ti, jm,
                                                       n0:n0 + pn],
                                                in_=asbs[jn][:pn, ti,
                                                             m0:m0 + mj],
                                                identity=ident[:pn, :pn])
                                atT = cat.tile([P, 2, 2, NQ], BF16,
                                               name="c_atT")
                                nc.scalar.copy(out=atT[:, :tn],
                                               in_=pt[:, :tn])
                                for ti in range(tn):
                                    for jm, (m0, mj) in enumerate(nch):
                                        nc.tensor.matmul(
                                            po[d0:d0 + D, ti, :],
                                            lhsT=v_res[:mj, tb + ti, jm,
                                                       hp * P + d0:
                                                       hp * P + d0 + D],
                                            rhs=atT[:mj, ti, jm, :],
                                            start=(jm == 0), stop=(jm == 1))
                            nc.scalar.copy(
                                out=o_fm[:, hp, tb * NF:(tb + tn) * NF],
                                in_=po[:, :tn, :])

                # ---------------- stage D: out_proj ----------------
                att_fm = res1.tile([P, KE, NKV], BF16, tag="tagB",
                                   name="att_fm")
                wo_sb = res1.tile([P, KE, E], BF16, tag="tagD", name="wo_sb")
                with tc.tile_pool(name="dob", bufs=1) as dbp, \
                     tc.tile_pool(name="dops", bufs=4, space="PSUM") as dpp:
                    for k in range(KE):
                        nc.sync.dma_start(out=wo_sb[:, k, :],
                                          in_=woT[k * P:(k + 1) * P, :])
                    bsb = dbp.tile([P, KE], F32, name="do_b")
                    nc.sync.dma_start(out=bsb, in_=_col_ap(bo, KE))
                    for n in range(NTILES):
                        n0 = n * NT
                        for m in range(KE):
                            ps = dpp.tile([P, NT], F32, name="do_ps")
                            for k in range(KE):
                                nc.tensor.matmul(
                                    ps[:, :],
                                    lhsT=wo_sb[:, k, m * P:(m + 1) * P],
                                    rhs=o_fm[:, k, n0:n0 + NT],
                                    start=(k == 0), stop=(k == KE - 1))
                            nc.scalar.activation(
                                out=att_fm[:, m, n0:n0 + NT], in_=ps[:, :],
                                func=AF.Identity, bias=bsb[:, m:m + 1])

                # ---------------- stage E1: mlpq GEMM1 ----------------
                def mlp_g1(wT_h, bias_h, src_fm, dst_nt, wpool, wtag,
                           stgpool, stgtag, tag):
                    with tc.tile_pool(name=tag + "b", bufs=1) as bp, \
                         tc.tile_pool(name=tag + "ps", bufs=4,
                                      space="PSUM") as pp:
                        bsb = bp.tile([P, KM], F32, name=tag + "_b")
                        nc.sync.dma_start(out=bsb, in_=_col_ap(bias_h, KM))
                        for mh in range(2):
                            wsb = wpool.tile([P, KE, MH], BF16, tag=wtag,
                                             name=tag + f"_w{mh}")
                            for k in range(KE):
                                nc.sync.dma_start(
                                    out=wsb[:, k, :],
                                    in_=wT_h[k * P:(k + 1) * P,
                                             mh * MH:(mh + 1) * MH])
                            for n in range(NTILES):
                                n0 = n * NT
                                for q in range(2):
                                    stg = stgpool.tile([P, 8, NT], BF16,
                                                       tag=stgtag, bufs=2,
                                                       name=tag + "_stg")
                                    for m8 in range(8):
                                        mi = mh * 16 + q * 8 + m8
                                        ps = pp.tile([P, NT], F32,
                                                     name=tag + "_ps")
                                        for k in range(KE):
                                            nc.tensor.matmul(
                                                ps[:, :],
                                                lhsT=wsb[:, k,
                                                         (q * 8 + m8) * P:
                                                         (q * 8 + m8 + 1) * P],
                                                rhs=src_fm[:, k,
                                                           n0:n0 + NT],
                                                start=(k == 0),
                                                stop=(k == KE - 1))
                                        nc.scalar.activation(
                                            out=stg[:, m8, :], in_=ps[:, :],
                                            func=AF.Gelu,
                                            bias=bsb[:, mi:mi + 1])
                                    nc.sync.dma_start(
                                        out=dst_nt[n][:, mh * 16 + q * 8:
                                                      mh * 16 + q * 8 + 8,
                                                      :],
                                        in_=stg)

                mlp_g1(w1qT, b1q, att_fm, h1q_nt, res1, "tagC", res1, "tagE1S", "e1")

            # =================== res2: stages E2..G2 ===================
            with tc.tile_pool(name="res2", bufs=1) as res2:
                # tagH: ln2t_fm (47 KB); tagW2: w2q -> w2 (64 KB)
                # tagW1: w1 halves (32 KB); quarters via scoped pools
                ln2t_fm = res2.tile([P, KE, NKV], BF16, tag="tagH",
                                    name="ln2t_fm")

                def ln_tm(stp, src, dst, gg, bb, p, apply_gb=True):
                    x3 = src.rearrange("p (n f) -> p n f", n=2)
                    st = stp.tile([P, 2, 6], F32, name="f_st")
                    for i in range(2):
                        nc.vector.bn_stats(out=st[:p, i, :], in_=x3[:p, i, :])
                    mv = stp.tile([P, 2], F32, name="f_mv")
                    nc.vector.bn_aggr(out=mv[:p], in_=st[:p])
                    rs = stp.tile([P, 1], F32, name="f_rs")
                    nc.scalar.activation(out=rs[:p], in_=mv[:p, 1:2],
                                         func=AF.Sqrt, bias=epst[:p])
                    nc.vector.reciprocal(out=rs[:p], in_=rs[:p])
                    nc.vector.tensor_scalar(
                        out=dst[:p], in0=src[:p], scalar1=mv[:p, 0:1],
                        scalar2=rs[:p], op0=ALU.subtract, op1=ALU.mult)
                    if apply_gb:
                        nc.vector.tensor_mul(dst[:p], dst[:p], gg[:p])
                        nc.vector.tensor_add(dst[:p], dst[:p], bb[:p])

                def mlp_g2(w_sb, src_nt, tag, epilogue, hq_first=None):
                    with tc.tile_pool(name=tag + "ps", bufs=3,
                                      space="PSUM") as pp:
                        for n in range(NTILES):
                            if n == 0 and hq_first is not None:
                                hqa, hqb = hq_first
                            else:
                                hqa = res2.tile([P, KM // 2, NT], BF16,
                                                tag="tagW1",
                                                name=tag + "_ha")
                                hqb = res2.tile([P, KM // 2, NT], BF16,
                                                tag="tagW1B",
                                                name=tag + "_hb")
                                for qtr in range(2):
                                    nc.sync.dma_start(
                                        out=hqa[:, qtr * 8:(qtr + 1) * 8,
                                                :],
                                        in_=src_nt[n][:,
                                                      qtr * 8:(qtr + 1) * 8,
                                                      :])
                                    nc.sync.dma_start(
                                        out=hqb[:, qtr * 8:(qtr + 1) * 8,
                                                :],
                                        in_=src_nt[n][:,
                                                      16 + qtr * 8:
                                                      16 + (qtr + 1) * 8,
                                                      :])
                            subs = [(0, P), (P, P), (2 * P, P),
                                    (3 * P, NT - 3 * P)]
                            for (s0, pr) in subs:
                                ps = pp.tile([P, E], F32, name=tag + "_ps")
                                for j in range(2):
                                    for k in range(KM):
                                        hk = hqa if k < 16 else hqb
                                        nc.tensor.matmul(
                                            ps[:pr, j * EH:(j + 1) * EH],
                                            lhsT=hk[:, k % 16, s0:s0 + pr],
                                            rhs=w_sb[:, k,
                                                     j * EH:(j + 1) * EH],
                                            start=(k == 0),
                                            stop=(k == KM - 1))
                                epilogue(ps, n * NT + s0, pr)

                # ---------------- stage E2 + F ----------------
                # load the first GEMM2 input tile before the 8 MB weight so
                # the first matmuls only wait for the leading weight chunks
                hq0a = res2.tile([P, KM // 2, NT], BF16, tag="tagW1",
                                 name="e2_h0a")
                hq0b = res2.tile([P, KM // 2, NT], BF16, tag="tagW1B",
                                 name="e2_h0b")
                for qtr in range(2):
                    nc.sync.dma_start(
                        out=hq0a[:, qtr * 8:(qtr + 1) * 8, :],
                        in_=h1q_nt[0][:, qtr * 8:(qtr + 1) * 8, :])
                    nc.sync.dma_start(
                        out=hq0b[:, qtr * 8:(qtr + 1) * 8, :],
                        in_=h1q_nt[0][:, 16 + qtr * 8:16 + (qtr + 1) * 8, :])
                w2q_sb = res2.tile([P, KM, E], BF16, tag="tagW2",
                                   name="w2q_sb")
                for k in range(KM):
                    nc.sync.dma_start(out=w2q_sb[:, k, :],
                                      in_=w2qT[k * P:(k + 1) * P, :])
                with tc.tile_pool(name="fg", bufs=1) as fg, \
                     tc.tile_pool(name="fst", bufs=4) as fst, \
                     tc.tile_pool(name="fw", bufs=2) as fwp, \
                     tc.tile_pool(name="fkv", bufs=2) as fkv, \
                     tc.tile_pool(name="ftp", bufs=2, space="PSUM") as ftp:
                    b2qb = fg.tile([P, E], BF16, name="f_b2q")
                    g1t = fg.tile([P, E], BF16, name="f_g1")
                    b1t = fg.tile([P, E], BF16, name="f_b1")
                    gkvt = fg.tile([P, E], BF16, name="f_gkv")
                    nc.gpsimd.dma_start(out=b2qb, in_=_bcast_ap(b2q, P))
                    nc.gpsimd.dma_start(out=g1t, in_=_bcast_ap(gres, P))
                    nc.gpsimd.dma_start(out=b1t, in_=_bcast_ap(bbres, P))
                    nc.gpsimd.dma_start(out=gkvt, in_=_bcast_ap(gkv, P))

                    def e2_epilogue(ps, r0, pr):
                        kvt = fkv.tile([P, E], BF16, name="f_kv")
                        nc.sync.dma_start(out=kvt[:pr],
                                          in_=kv_tm[r0:r0 + pr, :])
                        nc.vector.tensor_mul(kvt[:pr], kvt[:pr],
                                             gkvt[:pr])
                        qs = fwp.tile([P, E], F32, name="f_qs")
                        nc.vector.tensor_add(qs[:pr], ps[:pr, :], b2qb[:pr])
                        nc.vector.tensor_add(qs[:pr], qs[:pr], kvt[:pr])
                        y1 = fwp.tile([P, E], F32, name="f_y1")
                        ln_tm(fst, qs, y1, g1t, b1t, pr)
                        y2 = fwp.tile([P, E], BF16, name="f_y2")
                        ln_tm(fst, y1, y2, None, None, pr, apply_gb=False)
                        tp = ftp.tile([P, KE, P], BF16, name="f_tp")
                        for e in range(KE):
                            nc.tensor.transpose(
                                out=tp[:, e, :pr],
                                in_=y2[:pr, e * P:(e + 1) * P],
                                identity=ident[:pr, :pr])
                        nc.scalar.copy(out=ln2t_fm[:, :, r0:r0 + pr],
                                       in_=tp[:, :, :pr])

                    mlp_g2(w2q_sb, h1q_nt, "e2", e2_epilogue,
                           hq_first=(hq0a, hq0b))

                # ---------------- stage G1 ----------------
                mlp_g1(w1T, b1, ln2t_fm, h1_nt, res2, "tagW1", res2, "tagG1S", "g1")

                # ---------------- stage G2 ----------------
                w2_sb = res2.tile([P, KM, E], BF16, tag="tagW2", name="w2_sb")
                for k in range(KM):
                    nc.sync.dma_start(out=w2_sb[:, k, :],
                                      in_=w2T[k * P:(k + 1) * P, :])
                with tc.tile_pool(name="gg", bufs=1) as ggp, \
                     tc.tile_pool(name="gout", bufs=2) as gop:
                    b2b = ggp.tile([P, E], F32, name="g_b2")
                    nc.sync.dma_start(out=b2b, in_=_bcast_ap(b2, P))

                    def g2_epilogue(ps, r0, pr):
                        ot = gop.tile([P, E], F32, name="g_out")
                        nc.vector.tensor_add(ot[:pr], ps[:pr, :], b2b[:pr])
                        nc.sync.dma_start(out=out[r0:r0 + pr, :],
                                          in_=ot[:pr])

                    mlp_g2(w2_sb, h1_nt, "g2", g2_epilogue)

    nc.compile()
    return nc


_NC = None


def _get_nc():
    global _NC
    if _NC is None:
        _NC = build_nc()
    return _NC


def _prep_in_maps(inputs):
    f32 = lambda a: np.ascontiguousarray(np.asarray(a, dtype=np.float32))
    bf = lambda a: np.ascontiguousarray(
        np.asarray(a, dtype=np.float32).astype(ml_dtypes.bfloat16))
    x = f32(inputs["inputs"])                       # (B,HW,NF,E)
    ipw = f32(inputs["in_proj_w"])
    ipb = f32(inputs["in_proj_b"])
    wq, wk, wv = ipw[:E], ipw[E:2 * E], ipw[2 * E:]
    bq, bk_, bv_ = ipb[:E], ipb[E:2 * E], ipb[2 * E:]
    s = 1.0 / np.sqrt(np.float32(D))
    # fold kv-LN gain/bias into wk/wv/bk/bv and the mlpq residual path:
    # kv_ln = raw*g + b; k = kv_ln@wk.T+bk = raw@(wk*g).T + (bk + wk@b);
    # the qpre residual adds kv_ln = raw*g + b -> b folds into b2q, g is
    # applied on-chip via one broadcast multiply.
    gkv_v = f32(inputs["ln_kv_g"])
    bkv_v = f32(inputs["ln_kv_b"])
    bk_ = bk_ + wk @ bkv_v
    bv_ = bv_ + wv @ bkv_v
    wk = wk * gkv_v[None, :]
    wv = wv * gkv_v[None, :]
    # fold q-LN gain/bias into wq/bq (q_ln feeds only the q projection)
    gq_v = f32(inputs["ln_q_g"])
    bq_v = f32(inputs["ln_q_b"])
    wq_f = wq * gq_v[None, :]
    bq_f = bq + wq @ bq_v
    # fold ln2 gain/bias into mlp_w1/b1 (ln2 feeds only the final MLP)
    g2_v = f32(inputs["ln2_g"])
    b2_v = f32(inputs["ln2_b"])
    w1_f = f32(inputs["mlp_w1"]) * g2_v[None, :]
    b1_f = f32(inputs["mlp_b1"]) + f32(inputs["mlp_w1"]) @ b2_v
    shared = {
        "wqT": bf(wq_f.T * s), "wkT": bf(wk.T), "wvT": bf(wv.T),
        "woT": bf(f32(inputs["out_proj_w"]).T),
        "w1qT": bf(f32(inputs["mlpq_w1"]).T),
        "w2qT": bf(f32(inputs["mlpq_w2"]).T),
        "w1T": bf(w1_f.T),
        "w2T": bf(f32(inputs["mlp_w2"]).T),
        "bqs": f32(bq_f * s), "bk": f32(bk_), "bv": f32(bv_),
        "bo": f32(inputs["out_proj_b"]),
        "b1q": f32(inputs["mlpq_b1"]),
        "b2q": f32(f32(inputs["mlpq_b2"]) + bkv_v),
        "b1": f32(b1_f), "b2": f32(inputs["mlp_b2"]),
        "gq": f32(inputs["ln_q_g"]), "bbq": f32(inputs["ln_q_b"]),
        "gkv": f32(inputs["ln_kv_g"]), "bbkv": f32(inputs["ln_kv_b"]),
        "gres": f32(inputs["res_ln_g"]), "bbres": f32(inputs["res_ln_b"]),
        "gln2": f32(inputs["ln2_g"]), "bbln2": f32(inputs["ln2_b"]),
    }
    return [dict(shared, x=np.ascontiguousarray(x[b].reshape(NTOK, E)))
            for b in range(B)]


def _run(inputs, trace=False):
    from concourse.bass_utils import run_bass_kernel_spmd
    nc = _get_nc()
    in_maps = _prep_in_maps(inputs)
    res = run_bass_kernel_spmd(nc, in_maps, core_ids=list(range(B)),
                               trace=trace)
    outs = np.stack([r["out"].reshape(T, NF, E) for r in res.results])
    return outs, res


def kernel(**inputs) -> np.ndarray:
    outs, _ = _run(inputs, trace=False)
    return outs
